# revision 9
# baseline (speedup 1.0000x reference)
"""Trainium2 Bass kernel v3 for nn_Encoder_16956530884726.

Instruction-count-minimal wavefront. 8 cores data-parallel over batch
(16 rows/core); depth cells d=0..3 at partitions [32d, 32d+16). Cell d
processes t = w - 2d at wave w (slope-2 wavefront: h flows down 2 waves
later, shifted action 1 wave later).

This environment is instruction-issue bound (~25-100us/instruction
regardless of operand size; engines do not overlap; DMAs ~free), so v3
minimizes instruction count:
  - f32 matmuls everywhere (PE f32 is exact to ~1e-7) - no hi/lo splits
  - embedding x@W_emb computed on device (upload = raw x, 17MB vs 54MB)
  - LN scales via ACT Square+accum -> Sqrt -> DVE reciprocal, with all
    constant factors folded into activation scale/bias
  - gate/cand/action/s1 matmuls: 14 PE instr/wave total
  - every pure data movement is a DMA (SBUF<->SBUF free)
  - host streams (act/rst/dm0/sdm0 per wave) fully SBUF-resident
"""
import numpy as np

B = 128
BC = 16
H = 256
NCORES = 8
DEPTH = 4
EPS = 1e-5
LN1000 = float(np.log(np.float32(1000.0)))

_BUILD_CACHE = {}


def _as_ktiles(w):
    k, n = w.shape
    assert k == 256
    return np.ascontiguousarray(w.reshape(2, 128, n).transpose(1, 0, 2),
                                dtype=np.float32)


def build_nc_v3(L):
    import concourse.bacc as bacc
    import concourse.tile as tile
    from concourse import mybir
    from contextlib import ExitStack

    f32 = mybir.dt.float32
    i32 = mybir.dt.int32
    Alu = mybir.AluOpType
    Act = mybir.ActivationFunctionType

    NW = L + 2 * (DEPTH - 1)
    NCH = (L + 7) // 8          # 8-timestep chunks for row-layout embed
    LBP = NCH * 128             # padded x column count (t*16+b)
    SQG = float(np.sqrt(25.0 / 512.0))
    SQ1 = float(np.sqrt(25.0 / 768.0))
    SQC = 1.0 / 16.0

    nc = bacc.Bacc("TRN2", target_bir_lowering=False, debug=False,
                   num_devices=NCORES)
    P = nc.declare_dram_parameter
    XT = P("XT", [128, LBP], f32, isOutput=False)
    WEMB = P("WEMB", [128, 256], f32, isOutput=False)
    U2C = P("U2C", [128, 2, 512], f32, isOutput=False)
    U3C = P("U3C", [128, 2, 256], f32, isOutput=False)
    WGA = P("WGA", [128, 2, 512], f32, isOutput=False)
    WB = P("WB", [128, 2, 384], f32, isOutput=False)
    UA1 = P("UA1", [128, 2, 128], f32, isOutput=False)
    DWZ3 = P("DWZ3", [128, 2, 128], f32, isOutput=False)
    EYE = P("EYE", [128, 128], f32, isOutput=False)
    THR = P("THR", [128, 1], f32, isOutput=False)
    SELP = P("SELP", [128, 4], f32, isOutput=False)
    SCS = P("SCS", [128, NW, 5], f32, isOutput=False)
    OUT = P("OUT", [16, 256], f32, isOutput=True)
    BSUMS = P("BSUMS", [4, 1], f32, isOutput=True)

    with tile.TileContext(nc) as tc, ExitStack() as ctx:
        wp = ctx.enter_context(tc.tile_pool(name="weights", bufs=1))
        st = ctx.enter_context(tc.tile_pool(name="state", bufs=1))
        sc = ctx.enter_context(tc.tile_pool(name="scratch", bufs=3))
        psG = ctx.enter_context(tc.tile_pool(name="psG", bufs=1, space="PSUM"))
        psA = ctx.enter_context(tc.tile_pool(name="psA", bufs=1, space="PSUM"))
        psB = ctx.enter_context(tc.tile_pool(name="psB", bufs=1, space="PSUM"))
        psTR = ctx.enter_context(tc.tile_pool(name="psTR", bufs=2,
                                              space="PSUM"))
        psC = ctx.enter_context(tc.tile_pool(name="psC", bufs=1, space="PSUM"))

        # ---- weights / consts ----
        xt = wp.tile([128, LBP], f32, tag="xt")
        wemb = wp.tile([128, 256], f32, tag="wemb")
        u2c = wp.tile([128, 2, 512], f32, tag="u2c")
        u3c = wp.tile([128, 2, 256], f32, tag="u3c")
        wga = wp.tile([128, 2, 512], f32, tag="wga")
        wb = wp.tile([128, 2, 384], f32, tag="wb")
        ua1 = wp.tile([128, 2, 128], f32, tag="ua1")
        dwz3 = wp.tile([128, 2, 128], f32, tag="dwz3")
        eye = wp.tile([128, 128], f32, tag="eye")
        thr = wp.tile([128, 1], f32, tag="thr")
        sel = wp.tile([128, 4], f32, tag="sel")
        scs = wp.tile([128, NW, 5], f32, tag="scs")
        for t_, s_ in ((xt, XT), (wemb, WEMB), (u2c, U2C), (u3c, U3C),
                       (wga, WGA), (wb, WB), (ua1, UA1), (dwz3, DWZ3),
                       (eye, EYE), (thr, THR), (sel, SELP),
                       (scs, SCS)):
            nc.sync.dma_start(t_[:], s_[:])

        half = wp.tile([128, 512], f32, tag="half")
        nc.vector.memset(half[:], 0.5)
        c_eps25 = wp.tile([128, 1], f32, tag="c_eps25")
        nc.vector.memset(c_eps25[:], 25.0 * EPS)
        c_eps = wp.tile([128, 1], f32, tag="c_eps")
        nc.vector.memset(c_eps[:], EPS)

        # ---- persistent state ----
        h_t = [st.tile([128, 256], f32, tag=f"h{i}", name=f"h{i}")
               for i in range(2)]
        hT_t = [st.tile([128, 2, 128], f32, tag=f"hT{i}", name=f"hT{i}")
                for i in range(2)]
        ST = st.tile([128, 2], f32, tag="ST")          # [a_st, dmn/dmc]
        apdm = st.tile([128, 4, 2], f32, tag="apdm")   # ring: [a, dm]
        bsa = st.tile([128, 1], f32, tag="bsa")
        vpair = st.tile([128, 2], f32, tag="vpair")    # [25varg, 25var1]
        spair = st.tile([128, 2], f32, tag="spair")
        rpair = st.tile([128, 2], f32, tag="rpair")    # [a2, a1]
        s1g_s = st.tile([128, 512], f32, tag="s1g_s")
        s1c_s = st.tile([128, 256], f32, tag="s1c_s")
        xTb = st.tile([128, 2, 128], f32, tag="xTb")
        xet = st.tile([128, 2, LBP], f32, tag="xet")   # xe^T (h-partitioned)
        xer = st.tile([128, NCH * 256], f32, tag="xer")  # xe rows (8t chunks)
        for t_ in (*h_t, *hT_t, ST, apdm, bsa):
            nc.vector.memset(t_[:], 0.0)
        nc.vector.memset(xTb[:, :, 16:32], 0.0)

        # ---- embedding: xeT and xe-rows from XT ----
        embp = psG.tile([128, 512], f32, tag="embp")
        for kh in range(2):
            for c0 in range(0, LBP, 512):
                cw = min(512, LBP - c0)
                nc.tensor.matmul(embp[:, 0:cw], wemb[:, kh * 128:(kh + 1) * 128],
                                 xt[:, c0:c0 + cw], start=True, stop=True)
                nc.vector.tensor_copy(xet[:, kh, c0:c0 + cw], embp[:, 0:cw])
        for c in range(NCH):
            nc.tensor.matmul(embp[:, 0:256], xt[:, c * 128:(c + 1) * 128],
                             wemb[:], start=True, stop=True)
            nc.vector.tensor_copy(xer[:, c * 256:(c + 1) * 256],
                                  embp[:, 0:256])

        g_ps = psG.tile([128, 512], f32, tag="g_ps")
        s1a_ps = psA.tile([128, 512], f32, tag="s1a_ps")
        s1b_ps = psB.tile([128, 384], f32, tag="s1b_ps")

        # ---- tail block: compute wave wn's gate/s1/scale precursors ----
        def tail(wn):
            hTg = hT_t[wn % 2]        # h_seq(wn-1): gates h_tm1
            hTx = hT_t[(wn + 1) % 2]  # h_seq(wn-2): x input of cells 1-3
            # gates matmul for wave wn
            for k in range(2):
                nc.tensor.matmul(g_ps[:], hTg[:, k], u2c[:, k],
                                 start=(k == 0), stop=(k == 1))
            # xT assembly (cell0 <- host xeT col, cells 1-3 <- hT cols 0:96)
            t0 = min(wn, L - 1)
            nc.sync.dma_start(xTb[:, :, 0:16], xet[:, :, t0 * 16:t0 * 16 + 16])
            nc.sync.dma_start(xTb[:, :, 32:128], hTx[:, :, 0:96])
            # s1 matmuls: bank A (512) and bank B (s1c 256 | xa 128)
            for k in range(2):
                nc.tensor.matmul(s1a_ps[:], xTb[:, k], wga[:, k],
                                 start=(k == 0), stop=(k == 1))
            for k in range(2):
                nc.tensor.matmul(s1b_ps[:], xTb[:, k], wb[:, k],
                                 start=(k == 0), stop=False,
                                 skip_group_check=True)
            # variances (scales folded): vpair = [25*varg, 25*var1]
            sqg = sc.tile([128, 512], f32, tag="sqg")
            nc.scalar.activation(sqg[:], g_ps[:], Act.Square, scale=SQG,
                                 accum_out=vpair[:, 0:1])
            sqa = sc.tile([128, 512], f32, tag="sqa")
            vA = sc.tile([128, 1], f32, tag="vA")
            nc.scalar.activation(sqa[:], s1a_ps[:], Act.Square, scale=SQ1,
                                 accum_out=vA[:])
            sqb = sc.tile([128, 256], f32, tag="sqb")
            vB = sc.tile([128, 1], f32, tag="vB")
            nc.scalar.activation(sqb[:], s1b_ps[:, 0:256], Act.Square,
                                 scale=SQ1 / 5.0, accum_out=vB[:])
            nc.vector.tensor_tensor(out=vpair[:, 1:2], in0=vA[:], in1=vB[:],
                                    op=Alu.add)
            nc.scalar.activation(spair[:], vpair[:], Act.Sqrt,
                                 bias=c_eps25[:, 0:1])
            nc.vector.reciprocal(rpair[:], spair[:])
            nc.vector.scalar_tensor_tensor(out=s1g_s[:], in0=s1a_ps[:],
                                           scalar=rpair[:, 1:2], op0=Alu.mult,
                                           in1=half[:], op1=Alu.add)
            nc.vector.scalar_tensor_tensor(out=s1c_s[:], in0=s1b_ps[:, 0:256],
                                           scalar=rpair[:, 1:2], op0=Alu.mult,
                                           in1=half[:, 0:256], op1=Alu.bypass)

        tail(0)

        for w in range(NW):
            hp = h_t[w % 2]
            hn = h_t[(w + 1) % 2]
            hTc = hT_t[w % 2]
            hTn = hT_t[(w + 1) % 2]
            ap_t = apdm[:, (w - 1) % 4, 0:1]

            # finish action matmul: xa += hTc @ UA1
            for k in range(2):
                nc.tensor.matmul(s1b_ps[:, 256:384], hTc[:, k], ua1[:, k],
                                 start=False, stop=(k == 1),
                                 skip_group_check=True)

            # gates: s_rz = clip(g*a2 + s1g_s)
            s_rz = sc.tile([128, 512], f32, tag="s_rz")
            nc.vector.scalar_tensor_tensor(out=s_rz[:], in0=g_ps[:],
                                           scalar=rpair[:, 0:1], op0=Alu.mult,
                                           in1=s1g_s[:], op1=Alu.add)
            nc.vector.tensor_scalar(out=s_rz[:], in0=s_rz[:], scalar1=0.0,
                                    scalar2=1.0, op0=Alu.max, op1=Alu.min)
            # rh = r * hp ; transpose
            rh = sc.tile([128, 256], f32, tag="rh")
            nc.vector.tensor_tensor(out=rh[:], in0=s_rz[:, 256:512], in1=hp[:],
                                    op=Alu.mult)
            trR = psTR.tile([128, 256], f32, tag="tr")
            for k in range(2):
                nc.tensor.transpose(trR[:, k * 128:(k + 1) * 128],
                                    rh[:, k * 128:(k + 1) * 128], eye[:])
            rhT = sc.tile([128, 2, 128], f32, tag="rhT")
            nc.vector.tensor_copy(rhT[:], trR[:].rearrange("p (k c) -> p k c",
                                                           k=2))
            # cand matmul + LN + tanh
            c_ps = psC.tile([128, 256], f32, tag="c_ps")
            for k in range(2):
                nc.tensor.matmul(c_ps[:], rhT[:, k], u3c[:, k],
                                 start=(k == 0), stop=(k == 1))
            csq = sc.tile([128, 256], f32, tag="csq")
            vc = sc.tile([128, 1], f32, tag="vc")
            nc.scalar.activation(csq[:], c_ps[:], Act.Square, scale=SQC,
                                 accum_out=vc[:])
            stdc = sc.tile([128, 1], f32, tag="stdc")
            nc.scalar.activation(stdc[:], vc[:], Act.Sqrt,
                                 bias=c_eps[:, 0:1])
            a3 = sc.tile([128, 1], f32, tag="a3")
            nc.vector.reciprocal(a3[:], stdc[:])
            tpre = sc.tile([128, 256], f32, tag="tpre")
            nc.vector.scalar_tensor_tensor(out=tpre[:], in0=c_ps[:],
                                           scalar=a3[:, 0:1], op0=Alu.mult,
                                           in1=s1c_s[:], op1=Alu.add)
            T_t = sc.tile([128, 256], f32, tag="T_t")
            nc.scalar.activation(T_t[:], tpre[:], Act.Tanh)

            # action path
            u_t = sc.tile([128, 128], f32, tag="u_t")
            nc.vector.tensor_scalar(out=u_t[:], in0=s1b_ps[:, 256:384],
                                    scalar1=0.0, scalar2=None, op0=Alu.max)
            jj3 = sc.tile([128, 2, 128], f32, tag="jj3")
            u_bc = u_t[:].rearrange("p (o c) -> p o c",
                                    o=1).broadcast_to((128, 2, 128))
            nc.vector.tensor_tensor(out=jj3[:], in0=u_bc, in1=dwz3[:],
                                    op=Alu.mult)
            ddz = sc.tile([128, 2], f32, tag="ddz")
            nc.vector.tensor_reduce(out=ddz[:], in_=jj3[:],
                                    axis=mybir.AxisListType.X, op=Alu.add)
            t2 = sc.tile([128, 1], f32, tag="t2")
            nc.vector.tensor_tensor(out=t2[:], in0=ddz[:, 1:2], in1=thr[:],
                                    op=Alu.is_ge)
            action = sc.tile([128, 1], f32, tag="action")
            nc.vector.scalar_tensor_tensor(out=action[:], in0=ddz[:, 0:1],
                                           scalar=-2.0, op0=Alu.is_le,
                                           in1=t2[:], op1=Alu.max)
            nc.vector.tensor_tensor(out=action[:], in0=action[:], in1=ap_t,
                                    op=Alu.max)

            # scalar blend chain
            S = sc.tile([128, 2], f32, tag="S")        # [sdmx, dm_t]
            nc.vector.scalar_tensor_tensor(out=S[:, 0:1],
                                           in0=apdm[:, (w - 3) % 4, 1:2],
                                           scalar=scs[:, w, 4:5], op0=Alu.add,
                                           in1=scs[:, w, 2:3], op1=Alu.max)
            nc.vector.tensor_tensor(out=S[:, 1:2],
                                    in0=apdm[:, (w - 2) % 4, 1:2],
                                    in1=scs[:, w, 3:4], op=Alu.add)
            MD = sc.tile([128, 2], f32, tag="MD")      # [msk, dmg]
            nc.vector.tensor_tensor(out=MD[:], in0=S[:], in1=scs[:, w, 0:2],
                                    op=Alu.mult)
            u1 = sc.tile([128, 1], f32, tag="u1")
            nc.vector.tensor_scalar(out=u1[:], in0=ap_t, scalar1=-1.0,
                                    scalar2=1.0, op0=Alu.mult, op1=Alu.add)
            ub = sc.tile([128, 1], f32, tag="ub")
            nc.vector.tensor_tensor(out=ub[:], in0=u1[:], in1=MD[:, 1:2],
                                    op=Alu.mult)
            ma = sc.tile([128, 1], f32, tag="ma")
            nc.vector.tensor_tensor(out=ma[:], in0=action[:], in1=ST[:, 1:2],
                                    op=Alu.mult)
            both = sc.tile([128, 1], f32, tag="both")
            nc.vector.tensor_tensor(out=both[:], in0=ub[:], in1=ma[:],
                                    op=Alu.mult)
            nc.vector.tensor_tensor(out=bsa[:], in0=bsa[:], in1=both[:],
                                    op=Alu.add)
            sx = sc.tile([128, 1], f32, tag="sx")
            nc.vector.tensor_tensor(out=sx[:], in0=ub[:], in1=both[:],
                                    op=Alu.subtract)
            zz = sc.tile([128, 1], f32, tag="zz")
            nc.vector.tensor_tensor(out=zz[:], in0=both[:], in1=MD[:, 1:2],
                                    op=Alu.add)
            negq = sc.tile([128, 1], f32, tag="negq")  # qa2 = 1 + negq
            nc.vector.scalar_tensor_tensor(out=negq[:], in0=ma[:],
                                           scalar=MD[:, 1:2], op0=Alu.mult,
                                           in1=zz[:], op1=Alu.subtract)
            # a_st update must read old ST[:,1] (dmc) via ma above; now safe
            nc.vector.copy_predicated(ST[:, 0:1], MD[:, 0:1].bitcast(i32),
                                      action[:])
            nc.vector.scalar_tensor_tensor(out=ST[:, 1:2], in0=ma[:],
                                           scalar=ub[:, 0:1], op0=Alu.add,
                                           in1=both[:], op1=Alu.subtract)
            ship = sc.tile([128, 2], f32, tag="ship")
            nc.vector.tensor_tensor(out=ship[:], in0=ST[:],
                                    in1=scs[:, w, 0:2], op=Alu.mult)
            nc.sync.dma_start(apdm[32:128, w % 4, :], ship[0:96, :])

            # xb assembly
            xbt = sc.tile([128, 256], f32, tag="xbt")
            if w < 2:
                nc.vector.memset(xbt[:], 0.0)
            tcl = min(w, L - 1)
            nc.sync.dma_start(xbt[0:16, :],
                              xer[(tcl % 8) * 16:(tcl % 8) * 16 + 16,
                                  (tcl // 8) * 256:(tcl // 8) * 256 + 256])
            nc.sync.dma_start(xbt[32:128, :], h_t[(w + 1) % 2][0:96, :])

            # h blend: hn = both*hc + (1+negq)*hp + sx*xbt
            d1 = sc.tile([128, 256], f32, tag="d1")
            nc.vector.tensor_tensor(out=d1[:], in0=hp[:], in1=T_t[:],
                                    op=Alu.subtract)
            d2 = sc.tile([128, 256], f32, tag="d2")
            nc.vector.tensor_tensor(out=d2[:], in0=s_rz[:, 0:256], in1=d1[:],
                                    op=Alu.mult)
            hc = sc.tile([128, 256], f32, tag="hc")
            nc.vector.tensor_tensor(out=hc[:], in0=T_t[:], in1=d2[:],
                                    op=Alu.add)
            t1b = sc.tile([128, 256], f32, tag="t1b")
            nc.vector.scalar_tensor_tensor(out=t1b[:], in0=hp[:],
                                           scalar=negq[:, 0:1], op0=Alu.mult,
                                           in1=hp[:], op1=Alu.add)
            t2b = sc.tile([128, 256], f32, tag="t2b")
            nc.vector.scalar_tensor_tensor(out=t2b[:], in0=xbt[:],
                                           scalar=sx[:, 0:1], op0=Alu.mult,
                                           in1=t1b[:], op1=Alu.add)
            nc.vector.scalar_tensor_tensor(out=hn[:], in0=hc[:],
                                           scalar=both[:, 0:1], op0=Alu.mult,
                                           in1=t2b[:], op1=Alu.add)

            # h transpose
            trH = psTR.tile([128, 256], f32, tag="tr")
            for k in range(2):
                nc.tensor.transpose(trH[:, k * 128:(k + 1) * 128],
                                    hn[:, k * 128:(k + 1) * 128], eye[:])
            nc.vector.tensor_copy(hTn[:], trH[:].rearrange("p (k c) -> p k c",
                                                           k=2))

            if w + 1 < NW:
                tail(w + 1)

        # ---- outputs ----
        nc.sync.dma_start(OUT[:], h_t[NW % 2][96:112, :])
        bs_ps = psC.tile([128, 256], f32, tag="c_ps")
        nc.tensor.matmul(bs_ps[0:4, 0:1], sel[:], bsa[:], start=True,
                         stop=True)
        bs_sb = sc.tile([4, 1], f32, tag="bs_sb")
        nc.vector.tensor_copy(bs_sb[:], bs_ps[0:4, 0:1])
        nc.sync.dma_start(BSUMS[:], bs_sb[:])

    nc.finalize()
    return nc


# ===================== host side =====================

def _shared_weights(W, U, W_a1, U_a1, W_a2, W_emb):
    U2c = U[:, :512] - U[:, :512].mean(axis=1, keepdims=True)
    U3c = U[:, 512:] - U[:, 512:].mean(axis=1, keepdims=True)
    Wc = W - W.mean(axis=1, keepdims=True)
    sel = np.zeros((128, 4), np.float32)
    thr = np.full((128, 1), LN1000 + 1.0, np.float32)
    thr[96:128] = -1e30
    for d in range(4):
        sel[32 * d:32 * d + 16, d] = 1.0
    return {
        "WEMB": np.ascontiguousarray(W_emb, dtype=np.float32),
        "U2C": _as_ktiles(U2c),
        "U3C": _as_ktiles(U3c),
        "WGA": _as_ktiles(np.ascontiguousarray(Wc[:, 0:512])),
        "WB": _as_ktiles(np.concatenate([5.0 * Wc[:, 512:768], W_a1],
                                        axis=1)),
        "UA1": _as_ktiles(U_a1),
        "DWZ3": np.ascontiguousarray(np.stack([
            np.tile((W_a2[:, 0] - W_a2[:, 1])[None, :].astype(np.float32),
                    (128, 1)),
            np.tile(W_a2[:, 1][None, :].astype(np.float32), (128, 1))],
            axis=1)),
        "EYE": np.eye(128, dtype=np.float32),
        "THR": thr,
        "SELP": sel,
    }


def _build_scs(dm0_c, L, NW):
    """SCS [128, NW, 5] = [act, act, rst, dm0ch, sdm0ch]."""
    p = np.arange(128)
    d = p // 32
    bb = p % 32
    w = np.arange(NW)
    t = w[None, :] - 2 * d[:, None]                      # [128, NW]
    brow = bb < 16
    act = ((t >= 0) & (t < L) & brow[:, None]).astype(np.float32)
    rst = ((t == 0) & brow[:, None]).astype(np.float32)
    tc = np.clip(t, 0, L - 1)
    bc = np.minimum(bb, 15)
    dmv = dm0_c[tc, bc[:, None]]                         # [128, NW]
    dm0ch = np.where((d[:, None] == 0), dmv * act, 0.0)
    tm1 = np.clip(t - 1, 0, L - 1)
    sdmv = dm0_c[tm1, bc[:, None]]
    sdm0ch = np.where((d[:, None] == 0) & (t >= 1), sdmv * act, 0.0)
    scs = np.stack([act, act, rst, dm0ch, sdm0ch], axis=2)
    return np.ascontiguousarray(scs.astype(np.float32))


def make_in_maps(inputs, L):
    x = np.asarray(inputs["x"], np.float32)
    mask = np.asarray(inputs["mask"], np.float32)
    NW = L + 2 * (DEPTH - 1)
    NCH = (L + 7) // 8
    LBP = NCH * 128
    shared = _shared_weights(
        np.asarray(inputs["W"], np.float32),
        np.asarray(inputs["U"], np.float32),
        np.asarray(inputs["W_action_1"], np.float32),
        np.asarray(inputs["U_action_1"], np.float32),
        np.asarray(inputs["W_action_2"], np.float32),
        np.asarray(inputs["W_emb"], np.float32))
    dm0 = mask.T[:L]                                     # [L, B]
    in_maps = []
    for c in range(NCORES):
        bs = slice(c * BC, (c + 1) * BC)
        x_c = x[bs, :L, :]                               # [16, L, 128]
        xt = np.zeros((128, LBP), np.float32)
        xt[:, :L * 16] = x_c.transpose(2, 1, 0).reshape(128, L * 16)
        m = dict(shared)
        m["XT"] = np.ascontiguousarray(xt)
        m["SCS"] = _build_scs(np.ascontiguousarray(dm0[:, bs]), L, NW)
        in_maps.append(m)
    return in_maps


def kernel(**inputs):
    gammas = np.asarray(inputs["gammas"], np.float32)
    betas = np.asarray(inputs["betas"], np.float32)
    b_ = np.asarray(inputs["b"], np.float32)
    b_emb = np.asarray(inputs["b_emb"], np.float32)
    b_a1 = np.asarray(inputs["b_action_1"], np.float32)
    b_a2 = np.asarray(inputs["b_action_2"], np.float32)
    mask = np.asarray(inputs["mask"], np.float32)
    L = int(inputs["bucket_size"])

    ok = (np.all(gammas == 1.0) and np.all(betas == 0.0)
          and np.all(b_ == 0.0) and np.all(b_a1 == 0.0)
          and np.all(b_emb == 0.0) and np.all(mask == 1.0)
          and abs(float(b_a2[0]) - 1.0) < 1e-6
          and abs(float(b_a2[1]) + 1.0) < 1e-6 and L >= 2)
    if not ok:
        return _numpy_fallback(**inputs)

    try:
        from concourse.bass_utils import run_bass_kernel_spmd
        in_maps = make_in_maps(inputs, L)
        if L not in _BUILD_CACHE:
            _BUILD_CACHE[L] = build_nc_v3(L)
        nc = _BUILD_CACHE[L]
        res = run_bass_kernel_spmd(nc, in_maps, list(range(NCORES)))
        out = np.zeros((B, H), np.float32)
        gb = np.zeros(4, np.float64)
        for c in range(NCORES):
            out[c * BC:(c + 1) * BC] = res.results[c]["OUT"]
            gb += np.asarray(res.results[c]["BSUMS"][:, 0], np.float64)
        if gb[0] == 0.0 or gb[1] == 0.0:
            return _numpy_fallback(**inputs)
        if not np.all(np.isfinite(out)):
            return _numpy_fallback(**inputs)
        return out
    except Exception:
        import traceback
        traceback.print_exc()
        return _numpy_fallback(**inputs)


def _numpy_fallback(x, mask, bucket_size, W_emb, b_emb, W, U, b, W_action_1,
                    U_action_1, b_action_1, W_action_2, b_action_2,
                    gammas, betas):
    def ln(v, g, be):
        m = np.mean(v, axis=-1, keepdims=True)
        sd = np.sqrt(np.var(v, axis=-1, keepdims=True) + EPS)
        return g * ((v - m) / (sd + EPS)) + be

    L = int(bucket_size)
    dm0 = np.asarray(mask, np.float32).T[:L]
    xe = (np.asarray(x, np.float32) @ W_emb + b_emb).transpose(1, 0, 2)[:L]
    _, Bn = dm0.shape
    eos = dm0 * (1.0 - np.concatenate(
        [dm0[1:], np.zeros((1, Bn), np.float32)], 0))

    def horizontal(x_seq, ap_seq, dmask, llm):
        sdm = np.concatenate([np.ones((1, Bn), np.float32), dmask[:-1]], 0)
        sem = np.concatenate([np.zeros((1, Bn), np.float32), eos[:-1]], 0)
        xa = x_seq @ W_action_1 + b_action_1
        s1 = ln(x_seq @ W + b, gammas[0], betas[0])
        h = np.zeros((Bn, H), np.float32)
        a = np.zeros((Bn,), np.float32)
        dmc = np.zeros((Bn,), np.float32)
        h_seq = np.zeros((L, Bn, H), np.float32)
        a_seq = np.zeros((L, Bn), np.float32)
        dm_seq = np.zeros((L, Bn), np.float32)
        bs = 0.0
        for t in range(L):
            pol = np.maximum(xa[t] + h @ U_action_1, 0.0)
            pol2 = np.minimum(np.exp(pol @ W_action_2 + b_action_2), 1000.0)
            action = (pol2[:, 0] <= pol2[:, 1]).astype(np.float32)
            action = np.where(ap_seq[t] > 0, 1.0, action)
            action = np.where(llm > 0, 1.0, action)
            action = np.where(sem[t] > 0, 0.0, action)
            s2 = ln(h @ U[:, :512], gammas[1, :512], betas[1, :512])
            s = np.clip(0.2 * (s1[t][:, :512] + s2) + 0.5, 0, 1)
            z, r = s[:, :H], s[:, H:]
            h_cand = z * h + (1 - z) * np.tanh(
                s1[t][:, 512:] + ln((r * h) @ U[:, 512:], gammas[1, 512:],
                                    betas[1, 512:]))
            both = (1 - ap_seq[t]) * dmask[t] * action * dmc
            h_only = dmc * action * (ap_seq[t] + (1 - ap_seq[t]) * (1 - dmask[t]))
            x_only = dmask[t] * (1 - ap_seq[t]) * (1 - action + action * (1 - dmc))
            dmn = both + x_only + h_only
            h_new = both[:, None] * h_cand + h_only[:, None] * h + \
                x_only[:, None] * x_seq[t]
            a = np.where(sdm[t] > 0, action, a)
            h = np.where(dmask[t][:, None] > 0, h_new, h)
            dmc = dmn
            h_seq[t], a_seq[t], dm_seq[t] = h, a, dmn
            bs += float(both.sum())
        sa = np.concatenate([a_seq[1:], np.zeros((1, Bn), np.float32)], 0)
        return h_seq, sa, dm_seq, bs

    zeros_llm = np.zeros((Bn,), np.float32)
    ones_llm = np.ones((Bn,), np.float32)
    xc, apc, dmc, done = xe, np.zeros((L, Bn), np.float32), dm0, False
    for d in range(DEPTH - 1):
        hs, sa, ndm, bsum = horizontal(xc, apc, dmc, zeros_llm)
        if not done:
            xc, apc, dmc = hs, sa, ndm
        done = done or (bsum == 0)
    hs, _, _, _ = horizontal(xc, apc, dmc, ones_llm)
    return hs[-1]
